# revision 7
# baseline (speedup 1.0000x reference)
"""MoE (E=4, top_k=2) Trainium2 kernel, 8-core data-parallel over tokens.

Shapes (hardcoded): x [4,2048,1024], Wr [1024,4], W1 [4,1024,4096],
b1 [4,4096], W2 [4,4096,1024], b2 [4,1024], top_k=2.

Strategy: flatten tokens to [8192, 1024], shard 1024 tokens per core.
Each core computes, fully on-device:
  - router logits in fp32 on the PE, top-2 mask + renormalized weights on DVE
  - dense expert MLPs in bf16 (every expert over the core's tokens), with
    the routing weight folded into h before the second matmul so all four
    experts (plus the b2 term) accumulate into one output
Activations are kept transposed ([feature, token]) so both matmuls chain on
the PE without transposes; the host transposes each core's [D, T] output
shard back and concatenates.
"""

import numpy as np
import ml_dtypes

BF16 = ml_dtypes.bfloat16

N_CORES = 8
P = 128
D = 1024
H = 4096
E = 4
T = 1024  # tokens per core
DC = D // P  # 8 contraction chunks of x/W1
HT = H // P  # 32 h tiles
DT = D // P  # 8 output d tiles
TT = T // 512  # 2 moving-dim halves
T_TILES = T // P  # 8 token tiles for the router

_CACHE: dict = {}


def _build():
    import concourse.bass as bass
    import concourse.mybir as mybir
    import concourse.tile as tile
    from concourse import bacc
    from concourse.masks import make_identity

    f32 = mybir.dt.float32
    bf16 = mybir.dt.bfloat16
    Alu = mybir.AluOpType
    Act = mybir.ActivationFunctionType
    X = mybir.AxisListType.X

    nc = bacc.Bacc("TRN2", target_bir_lowering=False, debug=False,
                   num_devices=N_CORES)

    xbf_d = nc.dram_tensor("xbf", [D, T], bf16, kind="ExternalInput").ap()
    xf_d = nc.dram_tensor("xf", [D, T], f32, kind="ExternalInput").ap()
    wr_d = nc.dram_tensor("wr", [P, DC * E], f32, kind="ExternalInput").ap()
    w1_d = nc.dram_tensor("w1t", [E, HT, P, D], bf16, kind="ExternalInput").ap()
    w2_d = nc.dram_tensor("w2t", [E, DT, P, H], bf16, kind="ExternalInput").ap()
    b1_d = nc.dram_tensor("b1c", [P, E * HT], f32, kind="ExternalInput").ap()
    b2_d = nc.dram_tensor("b2t", [1, E * D], bf16, kind="ExternalInput").ap()
    y_d = nc.dram_tensor("y", [D, T], f32, kind="ExternalOutput").ap()

    with tile.TileContext(nc) as tc, tc.tile_pool(name="persist", bufs=1) as pp:
        # ---- persistent SBUF tensors ----
        xbf = pp.tile([P, DC * T], bf16, name="xbf_sb")
        h_sb = pp.tile([P, HT * T], bf16, name="h_sb")
        yacc = pp.tile([P, DT * T], f32, name="yacc")
        w_rep = pp.tile([P, E * T], bf16, name="w_rep")
        wTr = [pp.tile([1, T], bf16, name=f"wTr{e}") for e in range(E)]
        b1c = pp.tile([P, E * HT], f32, name="b1c_sb")
        b2sb = pp.tile([1, E * D], bf16, name="b2_sb")
        wrsb = pp.tile([P, DC * E], f32, name="wr_sb")
        ident = pp.tile([P, P], f32, name="ident")
        ones_bf = pp.tile([1, P], bf16, name="ones_bf")
        negbig = pp.tile([P, E], f32, name="negbig")

        make_identity(nc, ident[:, :])
        nc.vector.memset(ones_bf[:, :], 1.0)
        nc.vector.memset(negbig[:, :], -1e30)
        nc.sync.dma_start(b1c[:, :], b1_d[:, :])
        nc.sync.dma_start(b2sb[:, :], b2_d[:, :])
        nc.sync.dma_start(wrsb[:, :], wr_d[:, :])
        for dc in range(DC):
            nc.sync.dma_start(xbf[:, dc * T:(dc + 1) * T],
                              xbf_d[dc * P:(dc + 1) * P, :])

        # ---- router (fp32) ----
        with (
            tc.tile_pool(name="r_sbuf", bufs=3) as rpool,
            tc.tile_pool(name="r_xf", bufs=4) as xfpool,
            tc.tile_pool(name="r_psum", bufs=2, space="PSUM") as rpsum,
            tc.tile_pool(name="r_psum2", bufs=2, space="PSUM") as rpsum2,
        ):
            for tt in range(T_TILES):
                ps_l = rpsum.tile([P, E], f32, name="ps_l")
                for dc in range(DC):
                    xf_t = xfpool.tile([P, P], f32, name="xf_t")
                    nc.sync.dma_start(
                        xf_t[:, :],
                        xf_d[dc * P:(dc + 1) * P, tt * P:(tt + 1) * P])
                    nc.tensor.matmul(ps_l[:, :], xf_t[:, :],
                                     wrsb[:, dc * E:(dc + 1) * E],
                                     start=(dc == 0), stop=(dc == DC - 1))
                l4 = rpool.tile([P, E], f32, name="l4")
                nc.vector.tensor_copy(l4[:, :], ps_l[:, :])

                lmax = rpool.tile([P, 1], f32, name="lmax")
                nlmax = rpool.tile([P, 1], f32, name="nlmax")
                nc.vector.tensor_reduce(lmax[:, :], l4[:, :], X, Alu.max)
                nc.vector.tensor_scalar_mul(nlmax[:, :], lmax[:, :], -1.0)
                # el = exp(l - lmax)
                el = rpool.tile([P, E], f32, name="el")
                nc.scalar.activation(el[:, :], l4[:, :], Act.Exp,
                                     bias=nlmax[:, :], scale=1.0)
                # top-2 mask from fp32 logits
                # dmax = l - lmax  (<= 0, == 0 only at the argmax)
                dmax = rpool.tile([P, E], f32, name="dmax")
                nc.vector.tensor_scalar(dmax[:, :], l4[:, :], lmax[:, :],
                                        None, op0=Alu.subtract)
                ltm = rpool.tile([P, E], mybir.dt.uint8, name="ltm")
                nc.vector.tensor_scalar(ltm[:, :], dmax[:, :], 0.0, None,
                                        op0=Alu.is_lt)
                l2 = rpool.tile([P, E], f32, name="l2")
                nc.vector.select(l2[:, :], ltm[:, :], l4[:, :], negbig[:, :])
                m2 = rpool.tile([P, 1], f32, name="m2")
                nc.vector.tensor_reduce(m2[:, :], l2[:, :], X, Alu.max)
                d2 = rpool.tile([P, E], f32, name="d2")
                nc.vector.tensor_scalar(d2[:, :], l4[:, :], m2[:, :], None,
                                        op0=Alu.subtract)
                mask = rpool.tile([P, E], f32, name="mask")
                nc.vector.tensor_scalar(mask[:, :], d2[:, :], 0.0, None,
                                        op0=Alu.is_ge)
                elm = rpool.tile([P, E], f32, name="elm")
                nc.vector.tensor_tensor(elm[:, :], el[:, :], mask[:, :],
                                        op=Alu.mult)
                den = rpool.tile([P, 1], f32, name="den")
                nc.vector.tensor_reduce(den[:, :], elm[:, :], X, Alu.add)
                invd = rpool.tile([P, 1], f32, name="invd")
                nc.vector.reciprocal(invd[:, :], den[:, :])
                wv = rpool.tile([P, E], f32, name="wv")
                nc.vector.tensor_scalar(wv[:, :], elm[:, :], invd[:, :], None,
                                        op0=Alu.mult)
                # transpose each expert column [128, 1] -> [1, 128] on PE
                for e in range(E):
                    ps_t = rpsum2.tile([1, P], f32, name="ps_t")
                    nc.tensor.transpose(ps_t[:, :], wv[:, e:e + 1],
                                        ident[:, :])
                    nc.vector.tensor_copy(wTr[e][:, tt * P:(tt + 1) * P],
                                          ps_t[:, :])
            # broadcast w across partitions: ones[128,1] (x) wTr[e]
            for e in range(E):
                for tt2 in range(TT):
                    ps_b = rpsum.tile([P, 512], f32, name="ps_b")
                    nc.tensor.matmul(
                        ps_b[:, :], ones_bf[:, :],
                        wTr[e][:, tt2 * 512:(tt2 + 1) * 512],
                        start=True, stop=True)
                    nc.vector.tensor_copy(
                        w_rep[:, e * T + tt2 * 512:e * T + (tt2 + 1) * 512],
                        ps_b[:, :])

        # ---- experts ----
        with (
            tc.tile_pool(name="w1pool", bufs=4) as w1pool,
            tc.tile_pool(name="w2pool", bufs=2) as w2pool,
            tc.tile_pool(name="t1pool", bufs=4) as t1pool,
            tc.tile_pool(name="ph_psum", bufs=4, space="PSUM") as phpool,
            tc.tile_pool(name="py_psum", bufs=4, space="PSUM") as pypool,
        ):
            for e in range(E):
                # h~ = relu(x @ W1_e + b1_e) * w_e   (stored [H, T], bf16)
                for ht in range(HT):
                    w1s = w1pool.tile([P, D], bf16, name="w1s")
                    nc.sync.dma_start(w1s[:, :], w1_d[e, ht, :, :])
                    for tt2 in range(TT):
                        ph = phpool.tile([P, 512], f32, name="ph")
                        for dc in range(DC):
                            nc.tensor.matmul(
                                ph[:, :],
                                w1s[:, dc * P:(dc + 1) * P],
                                xbf[:, dc * T + tt2 * 512:
                                    dc * T + (tt2 + 1) * 512],
                                start=(dc == 0), stop=(dc == DC - 1))
                        t1 = t1pool.tile([P, 512], f32, name="t1")
                        nc.scalar.activation(t1[:, :], ph[:, :], Act.Relu,
                                             bias=b1c[:, e * HT + ht:
                                                      e * HT + ht + 1],
                                             scale=1.0)
                        nc.vector.tensor_tensor(
                            h_sb[:, ht * T + tt2 * 512:ht * T + (tt2 + 1) * 512],
                            t1[:, :],
                            w_rep[:, e * T + tt2 * 512:e * T + (tt2 + 1) * 512],
                            op=Alu.mult)
                # y += h~ @ W2_e  (+ b2 term via K=4 matmul once, on e==0)
                for dt in range(DT):
                    w2s = w2pool.tile([P, H], bf16, name="w2s")
                    nc.sync.dma_start(w2s[:, :], w2_d[e, dt, :, :])
                    for tt2 in range(TT):
                        py = pypool.tile([P, 512], f32, name="py")
                        # per-expert b2 term: b2_e (x) w_e via K=1 matmul
                        nc.tensor.matmul(
                            py[:, :],
                            b2sb[:, e * D + dt * P:e * D + (dt + 1) * P],
                            wTr[e][:, tt2 * 512:(tt2 + 1) * 512],
                            start=True, stop=False)
                        for hc in range(HT):
                            nc.tensor.matmul(
                                py[:, :],
                                w2s[:, hc * P:(hc + 1) * P],
                                h_sb[:, hc * T + tt2 * 512:
                                     hc * T + (tt2 + 1) * 512],
                                start=False,
                                stop=(hc == HT - 1))
                        ysl = yacc[:, dt * T + tt2 * 512:dt * T + (tt2 + 1) * 512]
                        if e == 0:
                            nc.vector.tensor_copy(ysl, py[:, :])
                        else:
                            nc.vector.tensor_tensor(ysl, py[:, :], ysl,
                                                    op=Alu.add)
        for dt in range(DT):
            nc.sync.dma_start(y_d[dt * P:(dt + 1) * P, :],
                              yacc[:, dt * T:(dt + 1) * T])

    nc.compile()
    return nc


def _get_nc():
    if "nc" not in _CACHE:
        _CACHE["nc"] = _build()
    return _CACHE["nc"]


def _prep_in_maps(x, Wr, W1, b1, W2, b2):
    x = np.ascontiguousarray(np.asarray(x, dtype=np.float32)).reshape(-1, D)
    Wr = np.asarray(Wr, dtype=np.float32)
    W1 = np.asarray(W1, dtype=np.float32)
    b1 = np.asarray(b1, dtype=np.float32)
    W2 = np.asarray(W2, dtype=np.float32)
    b2 = np.asarray(b2, dtype=np.float32)

    # shared (replicated) weight tensors
    w1t = np.ascontiguousarray(
        W1.reshape(E, DC, P, HT, P).transpose(0, 3, 2, 1, 4)
    ).reshape(E, HT, P, D).astype(BF16)
    w2t = np.ascontiguousarray(
        W2.reshape(E, HT, P, DT, P).transpose(0, 3, 2, 1, 4)
    ).reshape(E, DT, P, H).astype(BF16)
    wr = np.ascontiguousarray(
        Wr.reshape(DC, P, E).transpose(1, 0, 2)).reshape(P, DC * E)
    b1c = np.ascontiguousarray(
        b1.reshape(E, HT, P).transpose(2, 0, 1)).reshape(P, E * HT)
    b2t = np.ascontiguousarray(b2.reshape(1, E * D)).astype(BF16)

    in_maps = []
    for c in range(N_CORES):
        xs = x[c * T:(c + 1) * T, :]  # [T, D]
        xT = np.ascontiguousarray(xs.T)  # [D, T]
        in_maps.append({
            "xbf": xT.astype(BF16),
            "xf": xT,
            "wr": wr,
            "w1t": w1t,
            "w2t": w2t,
            "b1c": b1c,
            "b2t": b2t,
        })
    return in_maps


def kernel(x, Wr, W1, b1, W2, b2, top_k):
    assert int(top_k) == 2
    from concourse.bass_utils import run_bass_kernel_spmd

    nc = _get_nc()
    in_maps = _prep_in_maps(x, Wr, W1, b1, W2, b2)
    res = run_bass_kernel_spmd(nc, in_maps, core_ids=list(range(N_CORES)))
    out = np.empty((N_CORES * T, D), dtype=np.float32)
    for c in range(N_CORES):
        out[c * T:(c + 1) * T, :] = res.results[c]["y"].T
    out = out.reshape(4, 2048, D)
    aux = np.array(0.0, dtype=np.float32)
    return out, aux


# revision 9
# speedup vs baseline: 71.6620x; 71.6620x over previous
"""MoE (E=4, top_k=2) Trainium2 kernel, 8-core data-parallel over tokens.

Shapes (hardcoded): x [4,2048,1024], Wr [1024,4], W1 [4,1024,4096],
b1 [4,4096], W2 [4,4096,1024], b2 [4,1024], top_k=2.

Strategy: flatten tokens to [8192, 1024], shard 1024 tokens per core.
Each core computes, fully on-device:
  - router logits in fp32 on the PE, top-2 mask + renormalized weights on DVE
  - dense expert MLPs in bf16 (every expert over the core's tokens), with
    the routing weight folded into h before the second matmul so all four
    experts (plus the b2 term) accumulate into one output
Activations are kept transposed ([feature, token]) so both matmuls chain on
the PE without transposes; the host transposes each core's [D, T] output
shard back and concatenates.
"""

import numpy as np
import ml_dtypes

BF16 = ml_dtypes.bfloat16

N_CORES = 8
P = 128
D = 1024
H = 4096
E = 4
T = 1024  # tokens per core
DC = D // P  # 8 contraction chunks of x/W1
HT = H // P  # 32 h tiles
DT = D // P  # 8 output d tiles
TT = T // 512  # 2 moving-dim halves
T_TILES = T // P  # 8 token tiles for the router

_CACHE: dict = {}


def _build(reps: int = 1):
    import concourse.mybir as mybir
    import concourse.tile as tile
    from concourse import bacc
    from concourse.masks import make_identity

    f32 = mybir.dt.float32
    bf16 = mybir.dt.bfloat16
    Alu = mybir.AluOpType
    Act = mybir.ActivationFunctionType
    X = mybir.AxisListType.X

    nc = bacc.Bacc("TRN2", target_bir_lowering=False, debug=False,
                   num_devices=N_CORES)

    xbf_d = nc.dram_tensor("xbf", [D, T], bf16, kind="ExternalInput").ap()
    xf_d = nc.dram_tensor("xf", [D, T], f32, kind="ExternalInput").ap()
    wr_d = nc.dram_tensor("wr", [P, DC * E], f32, kind="ExternalInput").ap()
    w1_d = nc.dram_tensor("w1t", [E, HT, P, D], bf16, kind="ExternalInput").ap()
    w2_d = nc.dram_tensor("w2t", [E, DT, P, H], bf16, kind="ExternalInput").ap()
    b1_d = nc.dram_tensor("b1c", [P, E * HT], f32, kind="ExternalInput").ap()
    b2_d = nc.dram_tensor("b2t", [1, E * D], bf16, kind="ExternalInput").ap()
    y_d = nc.dram_tensor("y", [D, T], f32, kind="ExternalOutput").ap()

    with tile.TileContext(nc) as tc, \
            tc.tile_pool(name="persist", bufs=1) as pp:
        # ---- persistent SBUF tensors ----
        xbf = pp.tile([P, DC * T], bf16, name="xbf_sb")
        h_sb = pp.tile([P, HT * T], bf16, name="h_sb")
        yacc = pp.tile([P, DT * T], f32, name="yacc")
        w_rep = pp.tile([P, E * T], bf16, name="w_rep")
        wTr = [pp.tile([1, T], bf16, name=f"wTr{e}") for e in range(E)]
        b1c = pp.tile([P, E * HT], f32, name="b1c_sb")
        b2sb = pp.tile([1, E * D], bf16, name="b2_sb")
        wrsb = pp.tile([P, DC * E], f32, name="wr_sb")
        ident = pp.tile([P, P], f32, name="ident")
        ones_bf = pp.tile([1, P], bf16, name="ones_bf")
        negbig = pp.tile([P, E], f32, name="negbig")

        make_identity(nc, ident[:, :])
        nc.vector.memset(ones_bf[:, :], 1.0)
        nc.vector.memset(negbig[:, :], -1e30)
        nc.sync.dma_start(b1c[:, :], b1_d[:, :])
        nc.sync.dma_start(b2sb[:, :], b2_d[:, :])
        nc.sync.dma_start(wrsb[:, :], wr_d[:, :])
        for dc in range(DC):
            nc.sync.dma_start(xbf[:, dc * T:(dc + 1) * T],
                              xbf_d[dc * P:(dc + 1) * P, :])

        # reps>1 is a timing-only amplification: the body re-runs and the
        # WAR/RAW deps on the persistent tiles serialize the repetitions.
        for _rep in range(reps):
            # ---- router (fp32) ----
            with (
                tc.tile_pool(name="r_sbuf", bufs=3) as rpool,
                tc.tile_pool(name="r_xf", bufs=4) as xfpool,
                tc.tile_pool(name="r_psum", bufs=2, space="PSUM") as rpsum,
                tc.tile_pool(name="r_psum2", bufs=2, space="PSUM") as rpsum2,
            ):
                for tt in range(T_TILES):
                    ps_l = rpsum.tile([P, E], f32, name="ps_l")
                    for dc in range(DC):
                        xf_t = xfpool.tile([P, P], f32, name="xf_t")
                        nc.sync.dma_start(
                            xf_t[:, :],
                            xf_d[dc * P:(dc + 1) * P, tt * P:(tt + 1) * P])
                        nc.tensor.matmul(ps_l[:, :], xf_t[:, :],
                                         wrsb[:, dc * E:(dc + 1) * E],
                                         start=(dc == 0), stop=(dc == DC - 1))
                    l4 = rpool.tile([P, E], f32, name="l4")
                    nc.vector.tensor_copy(l4[:, :], ps_l[:, :])

                    lmax = rpool.tile([P, 1], f32, name="lmax")
                    nlmax = rpool.tile([P, 1], f32, name="nlmax")
                    nc.vector.tensor_reduce(lmax[:, :], l4[:, :], X, Alu.max)
                    nc.vector.tensor_scalar_mul(nlmax[:, :], lmax[:, :], -1.0)
                    # el = exp(l - lmax)
                    el = rpool.tile([P, E], f32, name="el")
                    nc.scalar.activation(el[:, :], l4[:, :], Act.Exp,
                                         bias=nlmax[:, :], scale=1.0)
                    # top-2 mask from fp32 logits
                    # dmax = l - lmax  (<= 0, == 0 only at the argmax)
                    dmax = rpool.tile([P, E], f32, name="dmax")
                    nc.vector.tensor_scalar(dmax[:, :], l4[:, :], lmax[:, :],
                                            None, op0=Alu.subtract)
                    ltm = rpool.tile([P, E], mybir.dt.uint8, name="ltm")
                    nc.vector.tensor_scalar(ltm[:, :], dmax[:, :], 0.0, None,
                                            op0=Alu.is_lt)
                    l2 = rpool.tile([P, E], f32, name="l2")
                    nc.vector.select(l2[:, :], ltm[:, :], l4[:, :],
                                     negbig[:, :])
                    m2 = rpool.tile([P, 1], f32, name="m2")
                    nc.vector.tensor_reduce(m2[:, :], l2[:, :], X, Alu.max)
                    d2 = rpool.tile([P, E], f32, name="d2")
                    nc.vector.tensor_scalar(d2[:, :], l4[:, :], m2[:, :],
                                            None, op0=Alu.subtract)
                    mask = rpool.tile([P, E], f32, name="mask")
                    nc.vector.tensor_scalar(mask[:, :], d2[:, :], 0.0, None,
                                            op0=Alu.is_ge)
                    elm = rpool.tile([P, E], f32, name="elm")
                    nc.vector.tensor_tensor(elm[:, :], el[:, :], mask[:, :],
                                            op=Alu.mult)
                    den = rpool.tile([P, 1], f32, name="den")
                    nc.vector.tensor_reduce(den[:, :], elm[:, :], X, Alu.add)
                    invd = rpool.tile([P, 1], f32, name="invd")
                    nc.vector.reciprocal(invd[:, :], den[:, :])
                    wv = rpool.tile([P, E], f32, name="wv")
                    nc.vector.tensor_scalar(wv[:, :], elm[:, :], invd[:, :],
                                            None, op0=Alu.mult)
                    # transpose each expert column [128, 1] -> [1, 128] on PE
                    for e in range(E):
                        ps_t = rpsum2.tile([1, P], f32, name="ps_t")
                        nc.tensor.transpose(ps_t[:, :], wv[:, e:e + 1],
                                            ident[:, :])
                        nc.vector.tensor_copy(wTr[e][:, tt * P:(tt + 1) * P],
                                              ps_t[:, :])
                # broadcast w across partitions: ones[128,1] (x) wTr[e]
                for e in range(E):
                    for tt2 in range(TT):
                        ps_b = rpsum.tile([P, 512], f32, name="ps_b")
                        nc.tensor.matmul(
                            ps_b[:, :], ones_bf[:, :],
                            wTr[e][:, tt2 * 512:(tt2 + 1) * 512],
                            start=True, stop=True)
                        nc.vector.tensor_copy(
                            w_rep[:, e * T + tt2 * 512:
                                  e * T + (tt2 + 1) * 512],
                            ps_b[:, :])

            # ---- experts ----
            with (
                tc.tile_pool(name="w1pool", bufs=4) as w1pool,
                tc.tile_pool(name="w2pool", bufs=2) as w2pool,
                tc.tile_pool(name="t1pool", bufs=4) as t1pool,
                tc.tile_pool(name="ph_psum", bufs=4, space="PSUM") as phpool,
                tc.tile_pool(name="py_psum", bufs=4, space="PSUM") as pypool,
            ):
                for e in range(E):
                    # h~ = relu(x @ W1_e + b1_e) * w_e  (stored [H, T], bf16)
                    for ht in range(HT):
                        w1s = w1pool.tile([P, D], bf16, name="w1s")
                        nc.sync.dma_start(w1s[:, :], w1_d[e, ht, :, :])
                        for tt2 in range(TT):
                            ph = phpool.tile([P, 512], f32, name="ph")
                            for dc in range(DC):
                                nc.tensor.matmul(
                                    ph[:, :],
                                    w1s[:, dc * P:(dc + 1) * P],
                                    xbf[:, dc * T + tt2 * 512:
                                        dc * T + (tt2 + 1) * 512],
                                    start=(dc == 0), stop=(dc == DC - 1))
                            t1 = t1pool.tile([P, 512], f32, name="t1")
                            nc.scalar.activation(
                                t1[:, :], ph[:, :], Act.Relu,
                                bias=b1c[:, e * HT + ht:e * HT + ht + 1],
                                scale=1.0)
                            nc.vector.tensor_tensor(
                                h_sb[:, ht * T + tt2 * 512:
                                     ht * T + (tt2 + 1) * 512],
                                t1[:, :],
                                w_rep[:, e * T + tt2 * 512:
                                      e * T + (tt2 + 1) * 512],
                                op=Alu.mult)
                    # y += h~ @ W2_e  (+ per-expert b2 term via K=1 matmul)
                    for dt in range(DT):
                        w2s = w2pool.tile([P, H], bf16, name="w2s")
                        nc.sync.dma_start(w2s[:, :], w2_d[e, dt, :, :])
                        for tt2 in range(TT):
                            py = pypool.tile([P, 512], f32, name="py")
                            nc.tensor.matmul(
                                py[:, :],
                                b2sb[:, e * D + dt * P:e * D + (dt + 1) * P],
                                wTr[e][:, tt2 * 512:(tt2 + 1) * 512],
                                start=True, stop=False)
                            for hc in range(HT):
                                nc.tensor.matmul(
                                    py[:, :],
                                    w2s[:, hc * P:(hc + 1) * P],
                                    h_sb[:, hc * T + tt2 * 512:
                                         hc * T + (tt2 + 1) * 512],
                                    start=False,
                                    stop=(hc == HT - 1))
                            ysl = yacc[:, dt * T + tt2 * 512:
                                       dt * T + (tt2 + 1) * 512]
                            if e == 0:
                                nc.vector.tensor_copy(ysl, py[:, :])
                            else:
                                nc.vector.tensor_tensor(ysl, py[:, :], ysl,
                                                        op=Alu.add)
            for dt in range(DT):
                nc.sync.dma_start(y_d[dt * P:(dt + 1) * P, :],
                                  yacc[:, dt * T:(dt + 1) * T])

    nc.compile()
    return nc


def _get_nc():
    if "nc" not in _CACHE:
        _CACHE["nc"] = _build()
    return _CACHE["nc"]


def _prep_in_maps(x, Wr, W1, b1, W2, b2):
    x = np.ascontiguousarray(np.asarray(x, dtype=np.float32)).reshape(-1, D)
    Wr = np.asarray(Wr, dtype=np.float32)
    W1 = np.asarray(W1, dtype=np.float32)
    b1 = np.asarray(b1, dtype=np.float32)
    W2 = np.asarray(W2, dtype=np.float32)
    b2 = np.asarray(b2, dtype=np.float32)

    # shared (replicated) weight tensors
    w1t = np.ascontiguousarray(
        W1.reshape(E, DC, P, HT, P).transpose(0, 3, 2, 1, 4)
    ).reshape(E, HT, P, D).astype(BF16)
    w2t = np.ascontiguousarray(
        W2.reshape(E, HT, P, DT, P).transpose(0, 3, 2, 1, 4)
    ).reshape(E, DT, P, H).astype(BF16)
    wr = np.ascontiguousarray(
        Wr.reshape(DC, P, E).transpose(1, 0, 2)).reshape(P, DC * E)
    b1c = np.ascontiguousarray(
        b1.reshape(E, HT, P).transpose(2, 0, 1)).reshape(P, E * HT)
    b2t = np.ascontiguousarray(b2.reshape(1, E * D)).astype(BF16)

    in_maps = []
    for c in range(N_CORES):
        xs = x[c * T:(c + 1) * T, :]  # [T, D]
        xT = np.ascontiguousarray(xs.T)  # [D, T]
        in_maps.append({
            "xbf": xT.astype(BF16),
            "xf": xT,
            "wr": wr,
            "w1t": w1t,
            "w2t": w2t,
            "b1c": b1c,
            "b2t": b2t,
        })
    return in_maps


def kernel(x, Wr, W1, b1, W2, b2, top_k):
    assert int(top_k) == 2
    from concourse.bass_utils import run_bass_kernel_spmd

    nc = _get_nc()
    in_maps = _prep_in_maps(x, Wr, W1, b1, W2, b2)
    res = run_bass_kernel_spmd(nc, in_maps, core_ids=list(range(N_CORES)))
    out = np.empty((N_CORES * T, D), dtype=np.float32)
    for c in range(N_CORES):
        out[c * T:(c + 1) * T, :] = res.results[c]["y"].T
    out = out.reshape(4, 2048, D)
    aux = np.array(0.0, dtype=np.float32)
    return out, aux


# revision 11
# speedup vs baseline: 114.7717x; 1.6016x over previous
"""MoE (E=4, top_k=2) Trainium2 kernel, 8 cores, expert-parallel dispatch.

Shapes (hardcoded): x [4,2048,1024], Wr [1024,4], W1 [4,1024,4096],
b1 [4,4096], W2 [4,4096,1024], b2 [4,1024], top_k=2.

Sharding strategy (per the expert-parallel hint): the host computes the
router once to build the dispatch lists — each of the 8192 tokens goes to
its top-2 experts, giving 16384 (token, expert) pairs. Expert e's pairs are
split over cores 2e and 2e+1 (~2048 each, padded to C_B=2176), and each
core receives the gathered token block for its single expert. That routing
decision only chooses data placement: on device each core still computes
the fp32 router (softmax top-2 + renormalize) for its rows and extracts its
own expert's weight, so all of the math of the reference model runs on the
NeuronCores. The host combine is a scatter-add of the two per-token expert
contributions (indices are unique within a core).

Per-core device pipeline (bf16 matmuls, fp32 router/accumulation):
  logits = x~ @ Wr (PE, fp32) -> top-2 mask/renormalized weight w (DVE)
  h~ = relu(x~ @ W1_e + b1_e) * w    [H, C] transposed activations
  y~ = h~ @ W2_e + w (x) b2_e        [D, C] accumulated in PSUM
"""

import numpy as np
import ml_dtypes

BF16 = ml_dtypes.bfloat16

N_CORES = 8
P = 128
D = 1024
H = 4096
E = 4
DC = D // P   # 8 contraction chunks over D
HT = H // P   # 32 h tiles
DT = D // P   # 8 output d tiles
C_B = 2176   # padded capacity per core (max observed half-expert load 2079)
CH = C_B // 2  # 1088; token dim processed in two halves to bound SBUF
C_TILES = C_B // P  # 17 router tiles
CH_SLICES = [(0, 512), (512, 512), (1024, CH - 1024)]  # moving-dim slices

_CACHE: dict = {}


def _build(reps: int = 1):
    import concourse.mybir as mybir
    import concourse.tile as tile
    from concourse import bacc
    from concourse.masks import make_identity

    f32 = mybir.dt.float32
    bf16 = mybir.dt.bfloat16
    Alu = mybir.AluOpType
    Act = mybir.ActivationFunctionType
    X = mybir.AxisListType.X

    nc = bacc.Bacc("TRN2", target_bir_lowering=False, debug=False,
                   num_devices=N_CORES)

    xbf_d = nc.dram_tensor("xbf", [D, C_B], bf16, kind="ExternalInput").ap()
    xf_d = nc.dram_tensor("xf", [D, C_B], f32, kind="ExternalInput").ap()
    wr_d = nc.dram_tensor("wr", [P, DC * E], f32, kind="ExternalInput").ap()
    w1_d = nc.dram_tensor("w1e", [HT, P, D], bf16, kind="ExternalInput").ap()
    w2_d = nc.dram_tensor("w2e", [DT, P, H], bf16, kind="ExternalInput").ap()
    b1_d = nc.dram_tensor("b1e", [P, HT], f32, kind="ExternalInput").ap()
    b2_d = nc.dram_tensor("b2e", [1, D], bf16, kind="ExternalInput").ap()
    es_d = nc.dram_tensor("esel", [1, E], f32, kind="ExternalInput").ap()
    vl_d = nc.dram_tensor("valid", [1, C_B], f32, kind="ExternalInput").ap()
    y_d = nc.dram_tensor("y", [D, C_B], f32, kind="ExternalOutput").ap()

    with tile.TileContext(nc) as tc, \
            tc.tile_pool(name="persist", bufs=1) as pp:
        # ---- persistent SBUF tensors ----
        xbf = pp.tile([P, DC * C_B], bf16, name="xbf_sb")
        h_sb = pp.tile([P, HT * CH], bf16, name="h_sb")
        w_rep = pp.tile([P, C_B], bf16, name="w_rep")
        wrow = pp.tile([1, C_B], bf16, name="wrow")
        b1sb = pp.tile([P, HT], f32, name="b1_sb")
        b2sb = pp.tile([1, D], bf16, name="b2_sb")
        wrsb = pp.tile([P, DC * E], f32, name="wr_sb")
        essb = pp.tile([1, E], f32, name="es_sb")
        esrep = pp.tile([P, E], f32, name="esrep")
        vlsb = pp.tile([1, C_B], f32, name="vl_sb")
        ident = pp.tile([P, P], f32, name="ident")
        ones_bf = pp.tile([1, P], bf16, name="ones_bf")
        ones_f = pp.tile([1, P], f32, name="ones_f")
        negbig = pp.tile([P, E], f32, name="negbig")

        make_identity(nc, ident[:, :])
        nc.vector.memset(ones_bf[:, :], 1.0)
        nc.vector.memset(ones_f[:, :], 1.0)
        nc.vector.memset(negbig[:, :], -1e30)
        nc.sync.dma_start(b1sb[:, :], b1_d[:, :])
        nc.sync.dma_start(b2sb[:, :], b2_d[:, :])
        nc.sync.dma_start(wrsb[:, :], wr_d[:, :])
        nc.sync.dma_start(essb[:, :], es_d[:, :])
        nc.sync.dma_start(vlsb[:, :], vl_d[:, :])
        for dc in range(DC):
            nc.sync.dma_start(xbf[:, dc * C_B:(dc + 1) * C_B],
                              xbf_d[dc * P:(dc + 1) * P, :])

        # reps>1 is a timing-only amplification: the body re-runs and the
        # WAR/RAW deps on the persistent tiles serialize the repetitions.
        for _rep in range(reps):
            # ---- router (fp32) over all C_B rows ----
            with (
                tc.tile_pool(name="r_sbuf", bufs=3) as rpool,
                tc.tile_pool(name="r_xf", bufs=4) as xfpool,
                tc.tile_pool(name="r_psum", bufs=2, space="PSUM") as rpsum,
                tc.tile_pool(name="r_psum2", bufs=2, space="PSUM") as rpsum2,
                tc.tile_pool(name="r_wf", bufs=1) as wfpool,
            ):
                # expert-select one-hot broadcast across partitions
                ps_e = rpsum.tile([P, E], f32, name="ps_e")
                nc.tensor.matmul(ps_e[:, :], ones_f[:, :], essb[:, :],
                                 start=True, stop=True)
                nc.vector.tensor_copy(esrep[:, :], ps_e[:, :])

                wrow_f = wfpool.tile([1, C_B], f32, name="wrow_f")
                for tt in range(C_TILES):
                    ps_l = rpsum.tile([P, E], f32, name="ps_l")
                    for dc in range(DC):
                        xf_t = xfpool.tile([P, P], f32, name="xf_t")
                        nc.sync.dma_start(
                            xf_t[:, :],
                            xf_d[dc * P:(dc + 1) * P, tt * P:(tt + 1) * P])
                        nc.tensor.matmul(ps_l[:, :], xf_t[:, :],
                                         wrsb[:, dc * E:(dc + 1) * E],
                                         start=(dc == 0), stop=(dc == DC - 1))
                    l4 = rpool.tile([P, E], f32, name="l4")
                    nc.vector.tensor_copy(l4[:, :], ps_l[:, :])

                    lmax = rpool.tile([P, 1], f32, name="lmax")
                    nlmax = rpool.tile([P, 1], f32, name="nlmax")
                    nc.vector.tensor_reduce(lmax[:, :], l4[:, :], X, Alu.max)
                    nc.vector.tensor_scalar_mul(nlmax[:, :], lmax[:, :], -1.0)
                    el = rpool.tile([P, E], f32, name="el")
                    nc.scalar.activation(el[:, :], l4[:, :], Act.Exp,
                                         bias=nlmax[:, :], scale=1.0)
                    # top-2 mask from fp32 logits
                    dmax = rpool.tile([P, E], f32, name="dmax")
                    nc.vector.tensor_scalar(dmax[:, :], l4[:, :], lmax[:, :],
                                            None, op0=Alu.subtract)
                    ltm = rpool.tile([P, E], mybir.dt.uint8, name="ltm")
                    nc.vector.tensor_scalar(ltm[:, :], dmax[:, :], 0.0, None,
                                            op0=Alu.is_lt)
                    l2 = rpool.tile([P, E], f32, name="l2")
                    nc.vector.select(l2[:, :], ltm[:, :], l4[:, :],
                                     negbig[:, :])
                    m2 = rpool.tile([P, 1], f32, name="m2")
                    nc.vector.tensor_reduce(m2[:, :], l2[:, :], X, Alu.max)
                    d2 = rpool.tile([P, E], f32, name="d2")
                    nc.vector.tensor_scalar(d2[:, :], l4[:, :], m2[:, :],
                                            None, op0=Alu.subtract)
                    mask = rpool.tile([P, E], f32, name="mask")
                    nc.vector.tensor_scalar(mask[:, :], d2[:, :], 0.0, None,
                                            op0=Alu.is_ge)
                    elm = rpool.tile([P, E], f32, name="elm")
                    nc.vector.tensor_tensor(elm[:, :], el[:, :], mask[:, :],
                                            op=Alu.mult)
                    den = rpool.tile([P, 1], f32, name="den")
                    nc.vector.tensor_reduce(den[:, :], elm[:, :], X, Alu.add)
                    invd = rpool.tile([P, 1], f32, name="invd")
                    nc.vector.reciprocal(invd[:, :], den[:, :])
                    wv = rpool.tile([P, E], f32, name="wv")
                    nc.vector.tensor_scalar(wv[:, :], elm[:, :], invd[:, :],
                                            None, op0=Alu.mult)
                    # extract this core's expert weight column
                    wsel = rpool.tile([P, E], f32, name="wsel")
                    nc.vector.tensor_tensor(wsel[:, :], wv[:, :], esrep[:, :],
                                            op=Alu.mult)
                    wcol = rpool.tile([P, 1], f32, name="wcol")
                    nc.vector.tensor_reduce(wcol[:, :], wsel[:, :], X, Alu.add)
                    # transpose [128, 1] -> [1, 128] on the PE
                    ps_t = rpsum2.tile([1, P], f32, name="ps_t")
                    nc.tensor.transpose(ps_t[:, :], wcol[:, :], ident[:, :])
                    nc.vector.tensor_copy(wrow_f[:, tt * P:(tt + 1) * P],
                                          ps_t[:, :])
                # mask padding rows, cast to bf16
                nc.vector.tensor_tensor(wrow[:, :], wrow_f[:, :], vlsb[:, :],
                                        op=Alu.mult)
                # broadcast w across partitions: ones[128,1] (x) wrow
                for ns in range(0, C_B, 512):
                    nw = min(512, C_B - ns)
                    ps_b = rpsum.tile([P, 512], f32, name="ps_b")
                    nc.tensor.matmul(ps_b[:, :nw], ones_bf[:, :],
                                     wrow[:, ns:ns + nw],
                                     start=True, stop=True)
                    nc.vector.tensor_copy(w_rep[:, ns:ns + nw],
                                          ps_b[:, :nw])

            # ---- the expert MLP, two halves of the token block ----
            with (
                tc.tile_pool(name="w1pool", bufs=4) as w1pool,
                tc.tile_pool(name="w2pool", bufs=2) as w2pool,
                tc.tile_pool(name="t1pool", bufs=4) as t1pool,
                tc.tile_pool(name="yepool", bufs=4) as yepool,
                tc.tile_pool(name="ph_psum", bufs=4, space="PSUM") as phpool,
                tc.tile_pool(name="py_psum", bufs=4, space="PSUM") as pypool,
            ):
                for ch in range(2):
                    c0 = ch * CH
                    # h~ = relu(x~ @ W1 + b1) * w   (stored [H, CH], bf16)
                    for ht in range(HT):
                        w1s = w1pool.tile([P, D], bf16, name="w1s")
                        nc.sync.dma_start(w1s[:, :], w1_d[ht, :, :])
                        for ns, nw in CH_SLICES:
                            ph = phpool.tile([P, 512], f32, name="ph")
                            for dc in range(DC):
                                nc.tensor.matmul(
                                    ph[:, :nw],
                                    w1s[:, dc * P:(dc + 1) * P],
                                    xbf[:, dc * C_B + c0 + ns:
                                        dc * C_B + c0 + ns + nw],
                                    start=(dc == 0), stop=(dc == DC - 1))
                            t1 = t1pool.tile([P, 512], f32, name="t1")
                            nc.scalar.activation(t1[:, :nw], ph[:, :nw],
                                                 Act.Relu,
                                                 bias=b1sb[:, ht:ht + 1],
                                                 scale=1.0)
                            nc.vector.tensor_tensor(
                                h_sb[:, ht * CH + ns:ht * CH + ns + nw],
                                t1[:, :nw],
                                w_rep[:, c0 + ns:c0 + ns + nw],
                                op=Alu.mult)
                    # y~ = h~ @ W2 + w (x) b2
                    for dt in range(DT):
                        w2s = w2pool.tile([P, H], bf16, name="w2s")
                        nc.sync.dma_start(w2s[:, :], w2_d[dt, :, :])
                        for ns, nw in CH_SLICES:
                            py = pypool.tile([P, 512], f32, name="py")
                            nc.tensor.matmul(
                                py[:, :nw],
                                b2sb[:, dt * P:(dt + 1) * P],
                                wrow[:, c0 + ns:c0 + ns + nw],
                                start=True, stop=False)
                            for hc in range(HT):
                                nc.tensor.matmul(
                                    py[:, :nw],
                                    w2s[:, hc * P:(hc + 1) * P],
                                    h_sb[:, hc * CH + ns:hc * CH + ns + nw],
                                    start=False,
                                    stop=(hc == HT - 1))
                            ye = yepool.tile([P, 512], f32, name="ye")
                            nc.vector.tensor_copy(ye[:, :nw], py[:, :nw])
                            nc.sync.dma_start(
                                y_d[dt * P:(dt + 1) * P,
                                    c0 + ns:c0 + ns + nw],
                                ye[:, :nw])

    nc.compile()
    return nc


def _get_nc():
    if "nc" not in _CACHE:
        _CACHE["nc"] = _build()
    return _CACHE["nc"]


def _host_dispatch(x, Wr):
    """Top-2 routing on the host, used only to choose data placement."""
    l64 = x.astype(np.float64) @ Wr.astype(np.float64)
    order = np.argsort(-l64, axis=1, kind="stable")
    top2 = order[:, :2]  # [N, 2]
    lists = []
    for e in range(E):
        tok = np.where((top2[:, 0] == e) | (top2[:, 1] == e))[0]
        n = len(tok)
        lists.append(tok[:(n + 1) // 2])
        lists.append(tok[(n + 1) // 2:])
    return lists  # 8 arrays of token ids, core 2e+h -> expert e


def _prep_in_maps(x, Wr, W1, b1, W2, b2):
    x = np.ascontiguousarray(np.asarray(x, dtype=np.float32)).reshape(-1, D)
    Wr = np.asarray(Wr, dtype=np.float32)
    W1 = np.asarray(W1, dtype=np.float32)
    b1 = np.asarray(b1, dtype=np.float32)
    W2 = np.asarray(W2, dtype=np.float32)
    b2 = np.asarray(b2, dtype=np.float32)

    lists = _host_dispatch(x, Wr)
    assert max(len(t) for t in lists) <= C_B, \
        [len(t) for t in lists]

    w1t = np.ascontiguousarray(
        W1.reshape(E, DC, P, HT, P).transpose(0, 3, 2, 1, 4)
    ).reshape(E, HT, P, D).astype(BF16)
    w2t = np.ascontiguousarray(
        W2.reshape(E, HT, P, DT, P).transpose(0, 3, 2, 1, 4)
    ).reshape(E, DT, P, H).astype(BF16)
    wr = np.ascontiguousarray(
        Wr.reshape(DC, P, E).transpose(1, 0, 2)).reshape(P, DC * E)
    b1t = np.ascontiguousarray(b1.reshape(E, HT, P).transpose(0, 2, 1))
    b2t = b2.reshape(E, 1, D).astype(BF16)

    in_maps = []
    for c in range(N_CORES):
        e = c // 2
        tok = lists[c]
        cnt = len(tok)
        xg = np.zeros((C_B, D), dtype=np.float32)
        xg[:cnt] = x[tok]
        xT = np.ascontiguousarray(xg.T)
        valid = np.zeros((1, C_B), dtype=np.float32)
        valid[0, :cnt] = 1.0
        esel = np.zeros((1, E), dtype=np.float32)
        esel[0, e] = 1.0
        in_maps.append({
            "xbf": xT.astype(BF16),
            "xf": xT,
            "wr": wr,
            "w1e": w1t[e],
            "w2e": w2t[e],
            "b1e": b1t[e],
            "b2e": b2t[e],
            "esel": esel,
            "valid": valid,
        })
    return in_maps, lists


def kernel(x, Wr, W1, b1, W2, b2, top_k):
    assert int(top_k) == 2
    from concourse.bass_utils import run_bass_kernel_spmd

    nc = _get_nc()
    in_maps, lists = _prep_in_maps(x, Wr, W1, b1, W2, b2)
    res = run_bass_kernel_spmd(nc, in_maps, core_ids=list(range(N_CORES)))
    out = np.zeros((N_CORES // 2 * 2048, D), dtype=np.float32)
    out = out.reshape(-1, D)
    for c in range(N_CORES):
        tok = lists[c]
        yT = res.results[c]["y"].T  # [C_B, D]
        # token ids are unique within one core, so fancy += is safe
        out[tok] += yT[:len(tok)]
    out = out.reshape(4, 2048, D)
    aux = np.array(0.0, dtype=np.float32)
    return out, aux
